# revision 49
# baseline (speedup 1.0000x reference)
"""MASKGCN Trainium2 kernel: 3-layer masked GCN over B=512 graphs of N=200 nodes.

Strategy
--------
Data-parallel over the batch: 64 graphs per NeuronCore, 8 cores, no collectives.

The reference model is LINEAR in the features (no activations), so the
readout folds through the last two layers (exact up to fp reassociation):
    mask = (E + E^T)/2 + I                        (host)
    A    = sigmoid(adj) * mask                    (host; adj is 0/1 so
                                                   sigmoid(adj) = c*adj + 0.5)
    out_g = mean_n(A(A(A(F W0) W1) W2))_n @ pw + pb
          = (1/N) colsum(A) @ A @ (A F W0) @ (W1 W2 pw) + pb
          = cs2 @ H1 @ Wq + pb
    with cs2 = colsum(A_g) @ A_g   (per-graph vector, host)
         Wq  = W1 @ W2 @ pw / N    (shared [256,2], host)
         H1  = A @ (F @ W0)        (first GCN layer, ON DEVICE)
This keeps the dense message-passing GEMMs on device and collapses only
the linear readout chain, the same fold the layer-3 colsum trick already
used. Measured end-to-end rel-norm error vs the fp32 reference: ~7e-4
(gate is 2e-2). All device tensors are fp16 (PE runs fp16 at 1
col/cycle vs fp32's 4; PSUM accumulation stays fp32).

Device dataflow (no on-chip transposes):
    S0  (normal  [node, h])  = matmul(lhsT=F^T slices, rhs=W0)
    H1t (transp. [h, node])  = matmul(lhsT=S0 slices,  rhs=A^T)
    TT  ([2, node])          = matmul(lhsT=Wq tiles,   rhs=H1t)
    og  ([2, 1])             = Vector multiply-reduce of TT against cs2
                               (accum_out) - Wq-stationary keeps the tail
                               LDWEIGHTS at 2 columns and no PE matmuls
                               wait on copies.

Schedule: four-graph software pipeline; S0 uses PSUM tag psA, H1 uses
psB (4 bufs each = all 8 banks), TT reuses ph1's freed columns. Every
PSUM evacuation hides under three partner graphs' matmul streams. The
host packs F^T|A^T row-wise so each graph loads in two 800B-per-row
DMAs, prefetched three quads ahead (dma_start dispatch costs ~360ns on
the sync queue, so the startup prefix carries only w0 + quad 0), and
~12 dummy matmuls on a zeroed tile burn the startup-DMA window so the
HAM clock gate opens the real stream at full PE clock.
"""

import os
import sys
import numpy as np

# concourse is normally pre-imported by the axon sitecustomize; these are
# fallbacks for environments where it is not on the default path.
if "concourse" not in sys.modules:
    try:
        import concourse  # noqa: F401
    except ImportError:
        for _p in ("/opt/trn_rl_repo", "/root/.axon_site/_ro/trn_rl_repo"):
            if os.path.isdir(_p) and _p not in sys.path:
                sys.path.append(_p)

B, N, IN_C, HID, OUT_C, N_VARS = 512, 200, 200, 256, 256, 2
N_CORES = 8
BPC = B // N_CORES  # graphs per core
P0 = 128
P1 = N - P0  # 72

# sigmoid(adj) = C_SIG * (adj + S_SIG) for adj in {0, 1}
C_SIG = float(1.0 / (1.0 + np.exp(-1.0)) - 0.5)  # 0.23105857863000487
S_SIG = float(0.5 / C_SIG)                       # 2.1639534137386535

_BUILD_CACHE = {}


def _build_nc(bpc, reps=1):
    """Build the per-core Bass program (SPMD: identical on all cores).

    reps>1 wraps the whole batch loop in a hardware For_i so the body runs
    `reps` times — benchmarking only (differencing cancels dispatch floor)."""
    import concourse.bacc as bacc
    import concourse.mybir as mybir
    import concourse.tile as tile
    from contextlib import ExitStack

    f32 = mybir.dt.float32
    f16 = mybir.dt.float16
    MULT = mybir.AluOpType.mult

    nc = bacc.Bacc(None, target_bir_lowering=False)
    # "fa0"/"fa1" pack F^T and host-prebuilt A^T = sigmoid(adj^T)*mask side
    # by side ([g, row, 0:200] = F^T row, [g, row, 200:400] = A^T row) so
    # each graph needs just two 800B-per-row DMAs. "wvt" carries
    # host-prebuilt colsum(A) packed [128, 2] per graph.
    fa0 = nc.declare_dram_parameter("fa0", [bpc, P0, 2 * N], f16, isOutput=False)
    fa1 = nc.declare_dram_parameter("fa1", [bpc, P1, 2 * N], f16, isOutput=False)
    csal = nc.declare_dram_parameter("csal", [2, bpc * N], f16, isOutput=False)
    w0 = nc.declare_dram_parameter("w0", [IN_C, HID], f16, isOutput=False)
    wp = nc.declare_dram_parameter("wp", [HID, N_VARS], f16, isOutput=False)
    out = nc.declare_dram_parameter("out", [N_VARS, bpc], f32, isOutput=True)

    with tile.TileContext(nc) as tc, ExitStack() as ctx:
        consts = ctx.enter_context(tc.tile_pool(name="consts", bufs=1))
        inp = ctx.enter_context(tc.tile_pool(name="inp", bufs=16))
        sp = ctx.enter_context(tc.tile_pool(name="sp", bufs=6))
        htp = ctx.enter_context(tc.tile_pool(name="htp", bufs=6))
        scrp = ctx.enter_context(tc.tile_pool(name="scrp", bufs=4))
        pstage = ctx.enter_context(
            tc.tile_pool(name="pstage", bufs=4, space="PSUM")
        )

        # ---- constants (loaded once) ----
        w0a = consts.tile([P0, HID], f16, tag="w0a")
        w0b = consts.tile([P1, HID], f16, tag="w0b")
        wpa = consts.tile([P0, N_VARS], f16, tag="wpa")
        wpb = consts.tile([P0, N_VARS], f16, tag="wpb")
        # ogall[c, g] accumulates og^T straight from the tail reduce.
        ogall = consts.tile([N_VARS, bpc], f32, tag="ogall")
        # csall[c, g*200:(g+1)*200] = colsum(A_g), duplicated on both rows.
        # Only w0 goes ahead of the first graph DMAs; the rest are issued
        # mid-first-quad.
        csall = consts.tile([2, bpc * N], f16, tag="csall")
        nc.sync.dma_start(w0a[:], w0[0:P0, :])
        nc.sync.dma_start(w0b[:], w0[P0:N, :])
        w0_t = (w0a, w0b)
        wp_t = (wpa, wpb)

        mslc = ((0, P0), (P0, P1))  # node-dim (offset, count) tiles

        # PE warmup: the HAM clock gate holds the PE at half clock until
        # it has seen ~3.4-5us of sustained matmul activity. The first
        # real matmul can't start until the startup DMAs land (~12us), so
        # burn that window with dummy matmuls on a zeroed scratch tile -
        # the real stream then opens at full clock. Sized to end at
        # data-ready: too many dummies delay the real stream (PE queue is
        # in-order).
        dummy_w = consts.tile([P0, 4 * P0], f16, tag="dummy_w")
        nc.vector.memzero(dummy_w[:])

        def emit_warmup():
            pd = pstage.tile([P0, 4 * P0], f32, tag="psA", name="psA")
            for _ in range(12):
                nc.tensor.matmul(
                    pd[:], dummy_w[:, 0:P0], dummy_w[:],
                    start=True, stop=True,
                )

        def emit_batch():
            state = {}

            def st_late_consts():
                # Issued after the first quad's input DMAs so they don't
                # delay the first matmul.
                nc.sync.dma_start(wpa[:], wp[0:P0, :])
                nc.sync.dma_start(wpb[:], wp[P0:HID, :])
                nc.sync.dma_start(csall[:], csal[:, :])

            def st_dma(g):
                t = {}
                fa0t = inp.tile([P0, 2 * N], f16, tag="fa0", name="fa0")
                fa1t = inp.tile([P1, 2 * N], f16, tag="fa1", name="fa1")
                nc.sync.dma_start(fa0t[:], fa0[g, :, :])
                nc.sync.dma_start(fa1t[:], fa1[g, :, :])
                t["fa"] = (fa0t, fa1t)
                state[g] = t

            def st_s0(g):
                # S0 = F @ W0 -> psum [node, 2*HID]; single fp16 evacuation
                t = state[g]
                ps0 = pstage.tile([P0, 2 * HID], f32, tag="psA", name="psA")
                fa = t["fa"]
                for j, (mo, mc) in enumerate(mslc):
                    for k in range(2):
                        nc.tensor.matmul(
                            ps0[0:mc, j * HID:(j + 1) * HID],
                            fa[k][:, mo:mo + mc],
                            w0_t[k][:],
                            start=(k == 0), stop=(k == 1),
                        )
                # s01 holds [nodes0:128 x hid | nodes128:200 x hid]; the
                # copy drags along garbage rows 72:128 of the second half.
                s01 = sp.tile([P0, 2 * HID], f16, tag="s01")
                nc.vector.tensor_copy(s01[:], ps0[:])
                t["s01"] = s01

            def st_h1(g):
                # H1^T = matmul(lhsT=S0 slices, rhs=A^T) -> psum [h, 2*N]
                t = state[g]
                ph1 = pstage.tile([P0, 2 * N], f32, tag="psB", name="psB")
                s01 = t["s01"]
                for j in range(2):  # h slice
                    for k, kc in ((0, P0), (1, P1)):  # node contraction tile
                        nc.tensor.matmul(
                            ph1[:, j * N:(j + 1) * N],
                            s01[0:kc, k * HID + j * P0:k * HID + (j + 1) * P0],
                            t["fa"][k][:, N:2 * N],
                            start=(k == 0), stop=(k == 1),
                        )
                h1 = htp.tile([P0, 2 * N], f16, tag="h1")
                nc.scalar.copy(h1[:], ph1[:, 0:2 * N])
                t["h1"] = h1
                t["ph1"] = ph1

            def st_tail(g):
                # Linear-model fold: out = cs2 @ H1 @ Wq with
                # cs2 = colsum(A)@A and Wq = W1@W2@pw/N precomputed on the
                # host, so layers 2-3 collapse into this readout.
                # T^T = Wq^T @ H1^T reuses ph1's freed low columns (the
                # rhs=h1 data dep orders the overwrite); Wq stationary
                # keeps LDWEIGHTS at 2 columns. og comes from an all-SBUF
                # fp16 Vector multiply-reduce straight into ogall.
                t = state[g]
                ph1 = t["ph1"]
                h1 = t["h1"]
                for k in range(2):  # hid contraction tile
                    nc.tensor.matmul(
                        ph1[0:N_VARS, 0:N],
                        wp_t[k][:],
                        h1[:, k * N:(k + 1) * N],
                        start=(k == 0), stop=(k == 1),
                    )
                s2t = scrp.tile([N_VARS, N], f16, tag="s2t")
                nc.scalar.copy(s2t[:], ph1[0:N_VARS, 0:N])
                scr = scrp.tile([N_VARS, N], f16, tag="scr")
                nc.vector.scalar_tensor_tensor(
                    scr[:], s2t[:], 1.0,
                    csall[:, g * N:(g + 1) * N],
                    op0=MULT, op1=MULT,
                    accum_out=ogall[:, g:g + 1],
                )
                del state[g]

            # Four-graph software pipeline: each stage's PSUM evacuation
            # hides under three partner graphs' matmul streams. S0/S1 share
            # PSUM tag psA, H1/H2/tail share psB (4 bufs each = 8 banks).
            # og matmuls run one quad late so their wait-on-copy never
            # blocks the PE queue.
            GRP = 4
            for q0 in range(0, bpc, GRP):
                G = list(range(q0, q0 + GRP))
                if q0 == 0:
                    # Minimal critical prefix: only quad 0's inputs go
                    # ahead of the first matmuls (each dma_start costs
                    # ~360ns of sync-queue dispatch).
                    for g in G:
                        st_dma(g)
                    emit_warmup()
                st_s0(G[0])
                st_s0(G[1])
                if q0 == 0:
                    # quads 1-2 outrank the wp/cs constants (not read
                    # until quad 0's tail) on the sync queue.
                    for g in range(GRP, 3 * GRP):
                        st_dma(g)
                    st_late_consts()
                st_s0(G[2])
                st_s0(G[3])
                for g in range(q0 + 3 * GRP, min(q0 + 4 * GRP, bpc)):
                    st_dma(g)
                for st in (st_h1, st_tail):
                    for g in G:
                        st(g)

        if reps > 1:
            with tc.For_i(0, reps, 1):
                emit_batch()
        else:
            emit_batch()

        nc.sync.dma_start(out[:], ogall[:])

    nc.compile()
    return nc


def _host_prep(adj, features, raw_edge_weight, W0, W1, W2, pw, pb):
    """Host-side prep: build A^T = sigmoid(adj^T)*mask and colsum(A) here
    (same bytes as shipping adj^T, but saves the on-device mask multiply),
    plus fp16 weight shards."""
    mask = ((raw_edge_weight + raw_edge_weight.T) * 0.5
            + np.eye(N, dtype=np.float32)).astype(np.float32)
    # Wq folds layers 2-3's weights + readout: W1 @ W2 @ pw / N
    wq = (W1.astype(np.float64) @ W2.astype(np.float64)
          @ pw.astype(np.float64) / float(N)).astype(np.float16)
    w0h = W0.astype(np.float16)
    # A^T[g] = (c*adj^T + 0.5) * mask  (mask is symmetric)
    adjt = np.ascontiguousarray(adj.transpose(0, 2, 1))
    at_all = ((np.float32(C_SIG) * adjt + np.float32(0.5)) * mask[None]
              ).astype(np.float16)
    ft16 = features.transpose(0, 2, 1).astype(np.float16)
    fa_all = np.concatenate([ft16, at_all], axis=2)  # [B, N, 2N]
    # cs2[g] = colsum(A_g) @ A_g (the layer-2/3 A-multiplies folded into
    # one host-precomputed vector), laid out [2, bpc*200] with the row
    # duplicated so the on-device reduce is partition-aligned with T^T.
    at32 = at_all.astype(np.float32)
    cs = at32.sum(axis=2)                                   # colsum(A) [B, N]
    # cs2[m'] = sum_m cs[m] * A[m, m'] = sum_m A^T[m', m] * cs[m]
    wv = np.matmul(at32, cs[:, :, None])[:, :, 0].astype(np.float16)
    in_maps = []
    for c in range(N_CORES):
        sl = slice(c * BPC, (c + 1) * BPC)
        in_maps.append({
            "fa0": np.ascontiguousarray(fa_all[sl, 0:P0, :]),
            "fa1": np.ascontiguousarray(fa_all[sl, P0:N, :]),
            "csal": np.ascontiguousarray(
                np.broadcast_to(wv[sl].reshape(1, BPC * N), (2, BPC * N))
            ),
            "w0": w0h,
            "wp": wq,
        })
    return in_maps


def _ensure_ntff_hook():
    """Wire the axon NTFF profile hook into antenv.axon_hooks if missing.

    The agent image's antenv package lacks axon_hooks, so bass_utils's
    trace path dies on import. trn_agent_boot has the ctypes hook
    implementation; expose it under the module name bass_utils expects.
    """
    import types

    try:
        from antenv.axon_hooks import get_axon_ntff_profile_hook  # noqa: F401
        return
    except ImportError:
        pass
    try:
        from trn_agent_boot.trn_boot import _ntff_profile_via_ctypes
        hook = _ntff_profile_via_ctypes("/opt/axon/libaxon_pjrt.so")
    except Exception:
        hook = None
    mod = types.ModuleType("antenv.axon_hooks")
    state = {"hook": hook}
    mod.get_axon_ntff_profile_hook = lambda: state["hook"]
    mod.set_axon_ntff_profile_hook = lambda h: state.__setitem__("hook", h)
    sys.modules["antenv.axon_hooks"] = mod
    import antenv

    antenv.axon_hooks = mod


def kernel(adj, features, raw_edge_weight, W0, W1, W2, pw, pb, _trace=False):
    from concourse.bass_utils import run_bass_kernel_spmd

    if _trace:
        _ensure_ntff_hook()

    adj = np.asarray(adj, dtype=np.float32)
    features = np.asarray(features, dtype=np.float32)
    raw_edge_weight = np.asarray(raw_edge_weight, dtype=np.float32)
    W0 = np.asarray(W0, dtype=np.float32)
    W1 = np.asarray(W1, dtype=np.float32)
    W2 = np.asarray(W2, dtype=np.float32)
    pw = np.asarray(pw, dtype=np.float32)
    pb = np.asarray(pb, dtype=np.float32)

    if "nc" not in _BUILD_CACHE:
        _BUILD_CACHE["nc"] = _build_nc(BPC)
    nc = _BUILD_CACHE["nc"]

    in_maps = _host_prep(adj, features, raw_edge_weight, W0, W1, W2, pw, pb)
    res = run_bass_kernel_spmd(
        nc, in_maps, core_ids=list(range(N_CORES)), trace=bool(_trace)
    )
    out = np.concatenate(
        [res.results[c]["out"].reshape(N_VARS, BPC).T for c in range(N_CORES)],
        axis=0,
    )
    out = out + pb[None, :].astype(np.float32)
    if _trace:
        return out, res
    return out


# revision 50
# speedup vs baseline: 1.0341x; 1.0341x over previous
"""MASKGCN Trainium2 kernel: 3-layer masked GCN over B=512 graphs of N=200 nodes.

Strategy
--------
Data-parallel over the batch: 64 graphs per NeuronCore, 8 cores, no collectives.

The reference model is LINEAR in the features (no activations), so the
readout folds through the last two layers (exact up to fp reassociation):
    mask = (E + E^T)/2 + I                        (host)
    A    = sigmoid(adj) * mask                    (host; adj is 0/1 so
                                                   sigmoid(adj) = c*adj + 0.5)
    out_g = mean_n(A(A(A(F W0) W1) W2))_n @ pw + pb
          = (1/N) colsum(A) @ A @ (A F W0) @ (W1 W2 pw) + pb
          = cs2 @ H1 @ Wq + pb
    with cs2 = colsum(A_g) @ A_g   (per-graph vector, host)
         Wq  = W1 @ W2 @ pw / N    (shared [256,2], host)
         H1  = A @ (F @ W0)        (first GCN layer, ON DEVICE)
This keeps the dense message-passing GEMMs on device and collapses only
the linear readout chain, the same fold the layer-3 colsum trick already
used. Measured end-to-end rel-norm error vs the fp32 reference: ~7e-4
(gate is 2e-2). All device tensors are fp16 (PE runs fp16 at 1
col/cycle vs fp32's 4; PSUM accumulation stays fp32).

Device dataflow (no on-chip transposes):
    S0  (normal  [node, h])  = matmul(lhsT=F^T slices, rhs=W0)
    H1t (transp. [h, node])  = matmul(lhsT=S0 slices,  rhs=A^T)
    TT  ([2, node])          = matmul(lhsT=Wq tiles,   rhs=H1t)
    og  ([2, 1])             = Vector multiply-reduce of TT against cs2
                               (accum_out) - Wq-stationary keeps the tail
                               LDWEIGHTS at 2 columns and no PE matmuls
                               wait on copies.

Schedule: four-graph software pipeline; S0 uses PSUM tag psA, H1 uses
psB (4 bufs each = all 8 banks), TT reuses ph1's freed columns. Every
PSUM evacuation hides under three partner graphs' matmul streams. The
host packs F^T|A^T row-wise so each graph loads in two 800B-per-row
DMAs, prefetched three quads ahead (dma_start dispatch costs ~360ns on
the sync queue, so the startup prefix carries only w0 + quad 0), and
~12 dummy matmuls on a zeroed tile burn the startup-DMA window so the
HAM clock gate opens the real stream at full PE clock.
"""

import os
import sys
import numpy as np

# concourse is normally pre-imported by the axon sitecustomize; these are
# fallbacks for environments where it is not on the default path.
if "concourse" not in sys.modules:
    try:
        import concourse  # noqa: F401
    except ImportError:
        for _p in ("/opt/trn_rl_repo", "/root/.axon_site/_ro/trn_rl_repo"):
            if os.path.isdir(_p) and _p not in sys.path:
                sys.path.append(_p)

B, N, IN_C, HID, OUT_C, N_VARS = 512, 200, 200, 256, 256, 2
N_CORES = 8
BPC = B // N_CORES  # graphs per core
P0 = 128
P1 = N - P0  # 72

# sigmoid(adj) = C_SIG * (adj + S_SIG) for adj in {0, 1}
C_SIG = float(1.0 / (1.0 + np.exp(-1.0)) - 0.5)  # 0.23105857863000487
S_SIG = float(0.5 / C_SIG)                       # 2.1639534137386535

_BUILD_CACHE = {}


def _build_nc(bpc, reps=1):
    """Build the per-core Bass program (SPMD: identical on all cores).

    reps>1 wraps the whole batch loop in a hardware For_i so the body runs
    `reps` times — benchmarking only (differencing cancels dispatch floor)."""
    import concourse.bacc as bacc
    import concourse.mybir as mybir
    import concourse.tile as tile
    from contextlib import ExitStack

    f32 = mybir.dt.float32
    f16 = mybir.dt.float16
    MULT = mybir.AluOpType.mult

    nc = bacc.Bacc(None, target_bir_lowering=False)
    # "fa0"/"fa1" pack F^T and host-prebuilt A^T = sigmoid(adj^T)*mask side
    # by side ([g, row, 0:200] = F^T row, [g, row, 200:400] = A^T row) so
    # each graph needs just two 800B-per-row DMAs. "wvt" carries
    # host-prebuilt colsum(A) packed [128, 2] per graph.
    fa0 = nc.declare_dram_parameter("fa0", [bpc, P0, 2 * N], f16, isOutput=False)
    fa1 = nc.declare_dram_parameter("fa1", [bpc, P1, 2 * N], f16, isOutput=False)
    csal = nc.declare_dram_parameter("csal", [2, bpc * N], f16, isOutput=False)
    w0 = nc.declare_dram_parameter("w0", [IN_C, HID], f16, isOutput=False)
    wp = nc.declare_dram_parameter("wp", [P0, 2 * N_VARS], f16, isOutput=False)
    out = nc.declare_dram_parameter("out", [N_VARS, bpc], f32, isOutput=True)

    with tile.TileContext(nc) as tc, ExitStack() as ctx:
        consts = ctx.enter_context(tc.tile_pool(name="consts", bufs=1))
        inp = ctx.enter_context(tc.tile_pool(name="inp", bufs=16))
        sp = ctx.enter_context(tc.tile_pool(name="sp", bufs=6))
        htp = ctx.enter_context(tc.tile_pool(name="htp", bufs=6))
        scrp = ctx.enter_context(tc.tile_pool(name="scrp", bufs=4))
        pstage = ctx.enter_context(
            tc.tile_pool(name="pstage", bufs=4, space="PSUM")
        )

        # ---- constants (loaded once) ----
        w0a = consts.tile([P0, HID], f16, tag="w0a")
        w0b = consts.tile([P1, HID], f16, tag="w0b")
        # Wq packed [128, 4]: cols 0:2 = rows 0:128, cols 2:4 = rows
        # 128:256 - one DMA with 8B descriptors instead of two with 4B.
        wpab = consts.tile([P0, 2 * N_VARS], f16, tag="wpab")
        # ogall[c, g] accumulates og^T straight from the tail reduce.
        ogall = consts.tile([N_VARS, bpc], f32, tag="ogall")
        # csall[c, g*200:(g+1)*200] = colsum(A_g), duplicated on both rows.
        # Only w0 goes ahead of the first graph DMAs; the rest are issued
        # mid-first-quad.
        csall = consts.tile([2, bpc * N], f16, tag="csall")
        nc.sync.dma_start(w0a[:], w0[0:P0, :])
        nc.sync.dma_start(w0b[:], w0[P0:N, :])
        w0_t = (w0a, w0b)
        wp_t = (wpab[:, 0:N_VARS], wpab[:, N_VARS:2 * N_VARS])

        mslc = ((0, P0), (P0, P1))  # node-dim (offset, count) tiles

        # PE warmup: the HAM clock gate holds the PE at half clock until
        # it has seen ~3.4-5us of sustained matmul activity. The first
        # real matmul can't start until the startup DMAs land (~12us), so
        # burn that window with dummy matmuls on a zeroed scratch tile -
        # the real stream then opens at full clock. Sized to end at
        # data-ready: too many dummies delay the real stream (PE queue is
        # in-order).
        dummy_w = consts.tile([P0, 4 * P0], f16, tag="dummy_w")
        nc.vector.memzero(dummy_w[:])

        def emit_warmup():
            pd = pstage.tile([P0, 4 * P0], f32, tag="psA", name="psA")
            for _ in range(12):
                nc.tensor.matmul(
                    pd[:], dummy_w[:, 0:P0], dummy_w[:],
                    start=True, stop=True,
                )

        def emit_batch():
            state = {}

            def st_late_consts():
                # Issued after the first quad's input DMAs so they don't
                # delay the first matmul.
                nc.sync.dma_start(wpab[:], wp[:, :])
                nc.sync.dma_start(csall[:], csal[:, :])

            def st_dma(g):
                t = {}
                fa0t = inp.tile([P0, 2 * N], f16, tag="fa0", name="fa0")
                fa1t = inp.tile([P1, 2 * N], f16, tag="fa1", name="fa1")
                nc.sync.dma_start(fa0t[:], fa0[g, :, :])
                nc.sync.dma_start(fa1t[:], fa1[g, :, :])
                t["fa"] = (fa0t, fa1t)
                state[g] = t

            def st_s0(g):
                # S0 = F @ W0 -> psum [node, 2*HID]; single fp16 evacuation
                t = state[g]
                ps0 = pstage.tile([P0, 2 * HID], f32, tag="psA", name="psA")
                fa = t["fa"]
                for j, (mo, mc) in enumerate(mslc):
                    for k in range(2):
                        nc.tensor.matmul(
                            ps0[0:mc, j * HID:(j + 1) * HID],
                            fa[k][:, mo:mo + mc],
                            w0_t[k][:],
                            start=(k == 0), stop=(k == 1),
                        )
                # s01 holds [nodes0:128 x hid | nodes128:200 x hid]; the
                # copy drags along garbage rows 72:128 of the second half.
                s01 = sp.tile([P0, 2 * HID], f16, tag="s01")
                nc.vector.tensor_copy(s01[:], ps0[:])
                t["s01"] = s01

            def st_h1(g):
                # H1^T = matmul(lhsT=S0 slices, rhs=A^T) -> psum [h, 2*N]
                t = state[g]
                ph1 = pstage.tile([P0, 2 * N], f32, tag="psB", name="psB")
                s01 = t["s01"]
                for j in range(2):  # h slice
                    for k, kc in ((0, P0), (1, P1)):  # node contraction tile
                        nc.tensor.matmul(
                            ph1[:, j * N:(j + 1) * N],
                            s01[0:kc, k * HID + j * P0:k * HID + (j + 1) * P0],
                            t["fa"][k][:, N:2 * N],
                            start=(k == 0), stop=(k == 1),
                        )
                h1 = htp.tile([P0, 2 * N], f16, tag="h1")
                nc.scalar.copy(h1[:], ph1[:, 0:2 * N])
                t["h1"] = h1
                t["ph1"] = ph1

            def st_tail(g):
                # Linear-model fold: out = cs2 @ H1 @ Wq with
                # cs2 = colsum(A)@A and Wq = W1@W2@pw/N precomputed on the
                # host, so layers 2-3 collapse into this readout.
                # T^T = Wq^T @ H1^T reuses ph1's freed low columns (the
                # rhs=h1 data dep orders the overwrite); Wq stationary
                # keeps LDWEIGHTS at 2 columns. og comes from an all-SBUF
                # fp16 Vector multiply-reduce straight into ogall.
                t = state[g]
                ph1 = t["ph1"]
                h1 = t["h1"]
                for k in range(2):  # hid contraction tile
                    nc.tensor.matmul(
                        ph1[0:N_VARS, 0:N],
                        wp_t[k],
                        h1[:, k * N:(k + 1) * N],
                        start=(k == 0), stop=(k == 1),
                    )
                s2t = scrp.tile([N_VARS, N], f16, tag="s2t")
                nc.scalar.copy(s2t[:], ph1[0:N_VARS, 0:N])
                scr = scrp.tile([N_VARS, N], f16, tag="scr")
                nc.vector.scalar_tensor_tensor(
                    scr[:], s2t[:], 1.0,
                    csall[:, g * N:(g + 1) * N],
                    op0=MULT, op1=MULT,
                    accum_out=ogall[:, g:g + 1],
                )
                del state[g]

            # Four-graph software pipeline: each stage's PSUM evacuation
            # hides under three partner graphs' matmul streams. S0/S1 share
            # PSUM tag psA, H1/H2/tail share psB (4 bufs each = 8 banks).
            # og matmuls run one quad late so their wait-on-copy never
            # blocks the PE queue.
            GRP = 4
            for q0 in range(0, bpc, GRP):
                G = list(range(q0, q0 + GRP))
                if q0 == 0:
                    # Minimal critical prefix: only quad 0's inputs go
                    # ahead of the first matmuls (each dma_start costs
                    # ~360ns of sync-queue dispatch).
                    for g in G:
                        st_dma(g)
                    emit_warmup()
                st_s0(G[0])
                st_s0(G[1])
                if q0 == 0:
                    # quads 1-2 outrank the wp/cs constants (not read
                    # until quad 0's tail) on the sync queue.
                    for g in range(GRP, 3 * GRP):
                        st_dma(g)
                    st_late_consts()
                st_s0(G[2])
                st_s0(G[3])
                for g in range(q0 + 3 * GRP, min(q0 + 4 * GRP, bpc)):
                    st_dma(g)
                for st in (st_h1, st_tail):
                    for g in G:
                        st(g)

        if reps > 1:
            with tc.For_i(0, reps, 1):
                emit_batch()
        else:
            emit_batch()

        nc.sync.dma_start(out[:], ogall[:])

    nc.compile()
    return nc


def _host_prep(adj, features, raw_edge_weight, W0, W1, W2, pw, pb):
    """Host-side prep: build A^T = sigmoid(adj^T)*mask and colsum(A) here
    (same bytes as shipping adj^T, but saves the on-device mask multiply),
    plus fp16 weight shards."""
    mask = ((raw_edge_weight + raw_edge_weight.T) * 0.5
            + np.eye(N, dtype=np.float32)).astype(np.float32)
    # Wq folds layers 2-3's weights + readout: W1 @ W2 @ pw / N
    wq = (W1.astype(np.float64) @ W2.astype(np.float64)
          @ pw.astype(np.float64) / float(N)).astype(np.float16)
    wq2 = np.ascontiguousarray(
        np.concatenate([wq[0:P0, :], wq[P0:HID, :]], axis=1)
    )
    w0h = W0.astype(np.float16)
    # A^T[g] = (c*adj^T + 0.5) * mask  (mask is symmetric)
    adjt = np.ascontiguousarray(adj.transpose(0, 2, 1))
    at_all = ((np.float32(C_SIG) * adjt + np.float32(0.5)) * mask[None]
              ).astype(np.float16)
    ft16 = features.transpose(0, 2, 1).astype(np.float16)
    fa_all = np.concatenate([ft16, at_all], axis=2)  # [B, N, 2N]
    # cs2[g] = colsum(A_g) @ A_g (the layer-2/3 A-multiplies folded into
    # one host-precomputed vector), laid out [2, bpc*200] with the row
    # duplicated so the on-device reduce is partition-aligned with T^T.
    at32 = at_all.astype(np.float32)
    cs = at32.sum(axis=2)                                   # colsum(A) [B, N]
    # cs2[m'] = sum_m cs[m] * A[m, m'] = sum_m A^T[m', m] * cs[m]
    wv = np.matmul(at32, cs[:, :, None])[:, :, 0].astype(np.float16)
    in_maps = []
    for c in range(N_CORES):
        sl = slice(c * BPC, (c + 1) * BPC)
        in_maps.append({
            "fa0": np.ascontiguousarray(fa_all[sl, 0:P0, :]),
            "fa1": np.ascontiguousarray(fa_all[sl, P0:N, :]),
            "csal": np.ascontiguousarray(
                np.broadcast_to(wv[sl].reshape(1, BPC * N), (2, BPC * N))
            ),
            "w0": w0h,
            "wp": wq2,
        })
    return in_maps


def _ensure_ntff_hook():
    """Wire the axon NTFF profile hook into antenv.axon_hooks if missing.

    The agent image's antenv package lacks axon_hooks, so bass_utils's
    trace path dies on import. trn_agent_boot has the ctypes hook
    implementation; expose it under the module name bass_utils expects.
    """
    import types

    try:
        from antenv.axon_hooks import get_axon_ntff_profile_hook  # noqa: F401
        return
    except ImportError:
        pass
    try:
        from trn_agent_boot.trn_boot import _ntff_profile_via_ctypes
        hook = _ntff_profile_via_ctypes("/opt/axon/libaxon_pjrt.so")
    except Exception:
        hook = None
    mod = types.ModuleType("antenv.axon_hooks")
    state = {"hook": hook}
    mod.get_axon_ntff_profile_hook = lambda: state["hook"]
    mod.set_axon_ntff_profile_hook = lambda h: state.__setitem__("hook", h)
    sys.modules["antenv.axon_hooks"] = mod
    import antenv

    antenv.axon_hooks = mod


def kernel(adj, features, raw_edge_weight, W0, W1, W2, pw, pb, _trace=False):
    from concourse.bass_utils import run_bass_kernel_spmd

    if _trace:
        _ensure_ntff_hook()

    adj = np.asarray(adj, dtype=np.float32)
    features = np.asarray(features, dtype=np.float32)
    raw_edge_weight = np.asarray(raw_edge_weight, dtype=np.float32)
    W0 = np.asarray(W0, dtype=np.float32)
    W1 = np.asarray(W1, dtype=np.float32)
    W2 = np.asarray(W2, dtype=np.float32)
    pw = np.asarray(pw, dtype=np.float32)
    pb = np.asarray(pb, dtype=np.float32)

    if "nc" not in _BUILD_CACHE:
        _BUILD_CACHE["nc"] = _build_nc(BPC)
    nc = _BUILD_CACHE["nc"]

    in_maps = _host_prep(adj, features, raw_edge_weight, W0, W1, W2, pw, pb)
    res = run_bass_kernel_spmd(
        nc, in_maps, core_ids=list(range(N_CORES)), trace=bool(_trace)
    )
    out = np.concatenate(
        [res.results[c]["out"].reshape(N_VARS, BPC).T for c in range(N_CORES)],
        axis=0,
    )
    out = out + pb[None, :].astype(np.float32)
    if _trace:
        return out, res
    return out


# revision 52
# speedup vs baseline: 1.0506x; 1.0159x over previous
"""MASKGCN Trainium2 kernel: 3-layer masked GCN over B=512 graphs of N=200 nodes.

Strategy
--------
Data-parallel over the batch: 64 graphs per NeuronCore, 8 cores, no collectives.

The reference model is LINEAR in the features (no activations), so the
readout folds through the last two layers (exact up to fp reassociation):
    mask = (E + E^T)/2 + I                        (host)
    A    = sigmoid(adj) * mask                    (host; adj is 0/1 so
                                                   sigmoid(adj) = c*adj + 0.5)
    out_g = mean_n(A(A(A(F W0) W1) W2))_n @ pw + pb
          = (1/N) colsum(A) @ A @ (A F W0) @ (W1 W2 pw) + pb
          = cs2 @ H1 @ Wq + pb
    with cs2 = colsum(A_g) @ A_g   (per-graph vector, host)
         Wq  = W1 @ W2 @ pw / N    (shared [256,2], host)
         H1  = A @ (F @ W0)        (first GCN layer, ON DEVICE)
This keeps the dense message-passing GEMMs on device and collapses only
the linear readout chain, the same fold the layer-3 colsum trick already
used. Measured end-to-end rel-norm error vs the fp32 reference: ~7e-4
(gate is 2e-2). All device tensors are fp16 (PE runs fp16 at 1
col/cycle vs fp32's 4; PSUM accumulation stays fp32).

Device dataflow (no on-chip transposes):
    S0  (normal  [node, h])  = matmul(lhsT=F^T slices, rhs=W0)
    H1t (transp. [h, node])  = matmul(lhsT=S0 slices,  rhs=A^T)
    TT  ([2, node])          = matmul(lhsT=Wq tiles,   rhs=H1t)
    og  ([2, 1])             = Vector multiply-reduce of TT against cs2
                               (accum_out) - Wq-stationary keeps the tail
                               LDWEIGHTS at 2 columns and no PE matmuls
                               wait on copies.

Schedule: four-graph software pipeline; S0 uses PSUM tag psA, H1 uses
psB (4 bufs each = all 8 banks), TT reuses ph1's freed columns. Every
PSUM evacuation hides under three partner graphs' matmul streams. The
host packs F^T|A^T row-wise so each graph loads in two 800B-per-row
DMAs, prefetched three quads ahead (dma_start dispatch costs ~360ns on
the sync queue, so the startup prefix carries only w0 + quad 0), and
~12 dummy matmuls on a zeroed tile burn the startup-DMA window so the
HAM clock gate opens the real stream at full PE clock.
"""

import os
import sys
import numpy as np

# concourse is normally pre-imported by the axon sitecustomize; these are
# fallbacks for environments where it is not on the default path.
if "concourse" not in sys.modules:
    try:
        import concourse  # noqa: F401
    except ImportError:
        for _p in ("/opt/trn_rl_repo", "/root/.axon_site/_ro/trn_rl_repo"):
            if os.path.isdir(_p) and _p not in sys.path:
                sys.path.append(_p)

B, N, IN_C, HID, OUT_C, N_VARS = 512, 200, 200, 256, 256, 2
N_CORES = 8
BPC = B // N_CORES  # graphs per core
P0 = 128
P1 = N - P0  # 72

# sigmoid(adj) = C_SIG * (adj + S_SIG) for adj in {0, 1}
C_SIG = float(1.0 / (1.0 + np.exp(-1.0)) - 0.5)  # 0.23105857863000487
S_SIG = float(0.5 / C_SIG)                       # 2.1639534137386535

_BUILD_CACHE = {}


def _build_nc(bpc, reps=1):
    """Build the per-core Bass program (SPMD: identical on all cores).

    reps>1 wraps the whole batch loop in a hardware For_i so the body runs
    `reps` times — benchmarking only (differencing cancels dispatch floor)."""
    import concourse.bacc as bacc
    import concourse.mybir as mybir
    import concourse.tile as tile
    from contextlib import ExitStack

    f32 = mybir.dt.float32
    f16 = mybir.dt.float16
    MULT = mybir.AluOpType.mult

    nc = bacc.Bacc(None, target_bir_lowering=False)
    # "fa0"/"fa1" pack F^T and host-prebuilt A^T = sigmoid(adj^T)*mask side
    # by side ([g, row, 0:200] = F^T row, [g, row, 200:400] = A^T row) so
    # each graph needs just two 800B-per-row DMAs. "wvt" carries
    # host-prebuilt colsum(A) packed [128, 2] per graph.
    fa0 = nc.declare_dram_parameter("fa0", [bpc, P0, 2 * N], f16, isOutput=False)
    fa1 = nc.declare_dram_parameter("fa1", [bpc, P1, 2 * N], f16, isOutput=False)
    csal = nc.declare_dram_parameter("csal", [2, bpc * N], f16, isOutput=False)
    w0 = nc.declare_dram_parameter("w0", [IN_C, HID], f16, isOutput=False)
    wp = nc.declare_dram_parameter("wp", [P0, 2 * N_VARS], f16, isOutput=False)
    out = nc.declare_dram_parameter("out", [N_VARS, bpc], f32, isOutput=True)

    with tile.TileContext(nc) as tc, ExitStack() as ctx:
        consts = ctx.enter_context(tc.tile_pool(name="consts", bufs=1))
        inp = ctx.enter_context(tc.tile_pool(name="inp", bufs=16))
        sp = ctx.enter_context(tc.tile_pool(name="sp", bufs=6))
        htp = ctx.enter_context(tc.tile_pool(name="htp", bufs=6))
        scrp = ctx.enter_context(tc.tile_pool(name="scrp", bufs=4))
        pstage = ctx.enter_context(
            tc.tile_pool(name="pstage", bufs=4, space="PSUM")
        )

        # ---- constants (loaded once) ----
        w0a = consts.tile([P0, HID], f16, tag="w0a")
        w0b = consts.tile([P1, HID], f16, tag="w0b")
        # Wq packed [128, 4]: cols 0:2 = rows 0:128, cols 2:4 = rows
        # 128:256 - one DMA with 8B descriptors instead of two with 4B.
        wpab = consts.tile([P0, 2 * N_VARS], f16, tag="wpab")
        # ogall[c, g] accumulates og^T straight from the tail reduce.
        ogall = consts.tile([N_VARS, bpc], f32, tag="ogall")
        # csall[c, g*200:(g+1)*200] = colsum(A_g), duplicated on both rows.
        # Only w0 goes ahead of the first graph DMAs; the rest are issued
        # mid-first-quad.
        csall = consts.tile([2, bpc * N], f16, tag="csall")
        nc.sync.dma_start(w0a[:], w0[0:P0, :])
        nc.sync.dma_start(w0b[:], w0[P0:N, :])
        w0_t = (w0a, w0b)
        wp_t = (wpab[:, 0:N_VARS], wpab[:, N_VARS:2 * N_VARS])

        mslc = ((0, P0), (P0, P1))  # node-dim (offset, count) tiles

        # PE warmup: the HAM clock gate holds the PE at half clock until
        # it has seen ~3.4-5us of sustained matmul activity. The first
        # real matmul can't start until the startup DMAs land (~12us), so
        # burn that window with dummy matmuls on a zeroed scratch tile -
        # the real stream then opens at full clock. Sized to end at
        # data-ready: too many dummies delay the real stream (PE queue is
        # in-order).
        dummy_w = consts.tile([P0, 4 * P0], f16, tag="dummy_w")
        nc.vector.memzero(dummy_w[:])

        def emit_warmup():
            pd = pstage.tile([P0, 4 * P0], f32, tag="psA", name="psA")
            for _ in range(12):
                nc.tensor.matmul(
                    pd[:], dummy_w[:, 0:P0], dummy_w[:],
                    start=True, stop=True,
                )

        def emit_batch():
            state = {}

            def st_late_consts():
                # Issued after the first quad's input DMAs so they don't
                # delay the first matmul.
                nc.sync.dma_start(wpab[:], wp[:, :])
                nc.sync.dma_start(csall[:], csal[:, :])

            def st_dma(g):
                t = {}
                fa0t = inp.tile([P0, 2 * N], f16, tag="fa0", name="fa0")
                fa1t = inp.tile([P1, 2 * N], f16, tag="fa1", name="fa1")
                nc.sync.dma_start(fa0t[:], fa0[g, :, :])
                nc.sync.dma_start(fa1t[:], fa1[g, :, :])
                t["fa"] = (fa0t, fa1t)
                state[g] = t

            def st_s0(g):
                # S0 = F @ W0 -> psum [node, 2*HID]; single fp16 evacuation
                t = state[g]
                ps0 = pstage.tile([P0, 2 * HID], f32, tag="psA", name="psA")
                fa = t["fa"]
                for j, (mo, mc) in enumerate(mslc):
                    for k in range(2):
                        nc.tensor.matmul(
                            ps0[0:mc, j * HID:(j + 1) * HID],
                            fa[k][:, mo:mo + mc],
                            w0_t[k][:],
                            start=(k == 0), stop=(k == 1),
                        )
                # s01 holds [nodes0:128 x hid | nodes128:200 x hid]; the
                # copy drags along garbage rows 72:128 of the second half.
                s01 = sp.tile([P0, 2 * HID], f16, tag="s01")
                nc.vector.tensor_copy(s01[:], ps0[:])
                t["s01"] = s01

            def st_h1(g):
                # H1^T = matmul(lhsT=S0 slices, rhs=A^T) -> psum [h, 2*N]
                t = state[g]
                ph1 = pstage.tile([P0, 2 * N], f32, tag="psB", name="psB")
                s01 = t["s01"]
                for j in range(2):  # h slice
                    for k, kc in ((0, P0), (1, P1)):  # node contraction tile
                        nc.tensor.matmul(
                            ph1[:, j * N:(j + 1) * N],
                            s01[0:kc, k * HID + j * P0:k * HID + (j + 1) * P0],
                            t["fa"][k][:, N:2 * N],
                            start=(k == 0), stop=(k == 1),
                        )
                h1 = htp.tile([P0, 2 * N], f16, tag="h1")
                nc.scalar.copy(h1[:], ph1[:, 0:2 * N])
                t["h1"] = h1
                t["ph1"] = ph1

            def st_tail(g):
                # Linear-model fold: out = cs2 @ H1 @ Wq with
                # cs2 = colsum(A)@A and Wq = W1@W2@pw/N precomputed on the
                # host, so layers 2-3 collapse into this readout.
                # T^T = Wq^T @ H1^T reuses ph1's freed low columns (the
                # rhs=h1 data dep orders the overwrite); Wq stationary
                # keeps LDWEIGHTS at 2 columns. og comes from an all-SBUF
                # fp16 Vector multiply-reduce straight into ogall.
                t = state[g]
                ph1 = t["ph1"]
                h1 = t["h1"]
                for k in range(2):  # hid contraction tile
                    nc.tensor.matmul(
                        ph1[0:N_VARS, 0:N],
                        wp_t[k],
                        h1[:, k * N:(k + 1) * N],
                        start=(k == 0), stop=(k == 1),
                    )
                s2t = scrp.tile([N_VARS, N], f16, tag="s2t")
                nc.scalar.copy(s2t[:], ph1[0:N_VARS, 0:N])
                scr = scrp.tile([N_VARS, N], f16, tag="scr")
                nc.vector.scalar_tensor_tensor(
                    scr[:], s2t[:], 1.0,
                    csall[:, g * N:(g + 1) * N],
                    op0=MULT, op1=MULT,
                    accum_out=ogall[:, g:g + 1],
                )
                del state[g]

            # Four-graph software pipeline: each stage's PSUM evacuation
            # hides under three partner graphs' matmul streams. S0/S1 share
            # PSUM tag psA, H1/H2/tail share psB (4 bufs each = 8 banks).
            # og matmuls run one quad late so their wait-on-copy never
            # blocks the PE queue.
            GRP = 4
            for q0 in range(0, bpc, GRP):
                G = list(range(q0, q0 + GRP))
                if q0 == 0:
                    # Minimal critical prefix: only quad 0's inputs go
                    # ahead of the first matmuls (each dma_start costs
                    # ~360ns of sync-queue dispatch).
                    for g in G:
                        st_dma(g)
                    emit_warmup()
                st_s0(G[0])
                st_s0(G[1])
                if q0 == 0:
                    # quads 1-2 outrank the wp/cs constants (not read
                    # until quad 0's tail) on the sync queue.
                    for g in range(GRP, 3 * GRP):
                        st_dma(g)
                    st_late_consts()
                st_s0(G[2])
                st_s0(G[3])
                for g in range(q0 + 3 * GRP, min(q0 + 4 * GRP, bpc)):
                    st_dma(g)
                for st in (st_h1, st_tail):
                    for g in G:
                        st(g)

        if reps > 1:
            with tc.For_i(0, reps, 1):
                emit_batch()
        else:
            emit_batch()

        nc.sync.dma_start(out[:], ogall[:])

    nc.compile()
    return nc


def _host_prep(adj, features, raw_edge_weight, W0, W1, W2, pw, pb):
    """Host-side prep: build A^T = sigmoid(adj^T)*mask and colsum(A) here
    (same bytes as shipping adj^T, but saves the on-device mask multiply),
    plus fp16 weight shards."""
    mask = ((raw_edge_weight + raw_edge_weight.T) * 0.5
            + np.eye(N, dtype=np.float32)).astype(np.float32)
    # Wq folds layers 2-3's weights + readout: W1 @ W2 @ pw / N
    wq = (W1.astype(np.float64) @ W2.astype(np.float64)
          @ pw.astype(np.float64) / float(N)).astype(np.float16)
    wq2 = np.ascontiguousarray(
        np.concatenate([wq[0:P0, :], wq[P0:HID, :]], axis=1)
    )
    w0h = W0.astype(np.float16)
    # A^T[g] = (c*adj^T + 0.5) * mask  (mask is symmetric)
    adjt = np.ascontiguousarray(adj.transpose(0, 2, 1))
    at_all = ((np.float32(C_SIG) * adjt + np.float32(0.5)) * mask[None]
              ).astype(np.float16)
    ft16 = features.transpose(0, 2, 1).astype(np.float16)
    fa_all = np.concatenate([ft16, at_all], axis=2)  # [B, N, 2N]
    # cs2[g] = colsum(A_g) @ A_g (the layer-2/3 A-multiplies folded into
    # one host-precomputed vector), laid out [2, bpc*200] with the row
    # duplicated so the on-device reduce is partition-aligned with T^T.
    at32 = at_all.astype(np.float32)
    cs = at32.sum(axis=2)                                   # colsum(A) [B, N]
    # cs2[m'] = sum_m cs[m] * A[m, m'] = sum_m A^T[m', m] * cs[m]
    wv = np.matmul(at32, cs[:, :, None])[:, :, 0].astype(np.float16)
    in_maps = []
    for c in range(N_CORES):
        sl = slice(c * BPC, (c + 1) * BPC)
        in_maps.append({
            "fa0": np.ascontiguousarray(fa_all[sl, 0:P0, :]),
            "fa1": np.ascontiguousarray(fa_all[sl, P0:N, :]),
            "csal": np.ascontiguousarray(
                np.broadcast_to(wv[sl].reshape(1, BPC * N), (2, BPC * N))
            ),
            "w0": w0h,
            "wp": wq2,
        })
    return in_maps


def _ensure_ntff_hook():
    """Wire the axon NTFF profile hook into antenv.axon_hooks if missing.

    The agent image's antenv package lacks axon_hooks, so bass_utils's
    trace path dies on import. trn_agent_boot has the ctypes hook
    implementation; expose it under the module name bass_utils expects.
    """
    import types

    try:
        from antenv.axon_hooks import get_axon_ntff_profile_hook  # noqa: F401
        return
    except ImportError:
        pass
    try:
        from trn_agent_boot.trn_boot import _ntff_profile_via_ctypes
        hook = _ntff_profile_via_ctypes("/opt/axon/libaxon_pjrt.so")
    except Exception:
        hook = None
    mod = types.ModuleType("antenv.axon_hooks")
    state = {"hook": hook}
    mod.get_axon_ntff_profile_hook = lambda: state["hook"]
    mod.set_axon_ntff_profile_hook = lambda h: state.__setitem__("hook", h)
    sys.modules["antenv.axon_hooks"] = mod
    import antenv

    antenv.axon_hooks = mod


def kernel(adj, features, raw_edge_weight, W0, W1, W2, pw, pb, _trace=False):
    from concourse.bass_utils import run_bass_kernel_spmd

    if _trace:
        _ensure_ntff_hook()

    adj = np.asarray(adj, dtype=np.float32)
    features = np.asarray(features, dtype=np.float32)
    raw_edge_weight = np.asarray(raw_edge_weight, dtype=np.float32)
    W0 = np.asarray(W0, dtype=np.float32)
    W1 = np.asarray(W1, dtype=np.float32)
    W2 = np.asarray(W2, dtype=np.float32)
    pw = np.asarray(pw, dtype=np.float32)
    pb = np.asarray(pb, dtype=np.float32)

    if "nc" not in _BUILD_CACHE:
        _BUILD_CACHE["nc"] = _build_nc(BPC)
    nc = _BUILD_CACHE["nc"]

    in_maps = _host_prep(adj, features, raw_edge_weight, W0, W1, W2, pw, pb)
    res = run_bass_kernel_spmd(
        nc, in_maps, core_ids=list(range(N_CORES)), trace=bool(_trace)
    )
    out = np.concatenate(
        [res.results[c]["out"].reshape(N_VARS, BPC).T for c in range(N_CORES)],
        axis=0,
    )
    out = out + pb[None, :].astype(np.float32)
    if _trace:
        return out, res
    return out
